# revision 10
# baseline (speedup 1.0000x reference)
"""Multi-head causal attention (B=2, S=2048, D=2048, H=16) on 8 TRN2 NeuronCores.

Sharding: tensor-parallel over heads — each core computes 2 heads end-to-end
(QKV projection columns, RoPE, causal attention, wo projection rows). The wo
partial outputs are summed on the host (row-parallel matmul unshard).

Device layouts (per core):
  xT   [2048, 4096]  x transposed: model-dim on partitions, tokens free
  Q/K  computed transposed [d, tok] so attention needs no activations transpose:
       S^T[k,q] = (K^T slice).T-matmul(Q^T slice); softmax runs on ACT with the
       row sum realized as a ones-vector matmul on the PE; P^T feeds P@V
       directly, producing ctx^T[d, tok] which is exactly the lhsT the wo
       projection needs.
  RoPE pair-swap (interleaved rotary) is a 128x128 permutation matmul on PE.
"""

import math
import sys

sys.path.insert(0, "/opt/trn_rl_repo")

import numpy as np

B = 2
S = 2048
D = 2048
H = 16
HD = 128
NCORES = 8
HLOC = H // NCORES      # heads per core
CW = HLOC * HD          # per-core projection width (256)
NTOK = B * S            # 4096
CN = 512                # phase-1 token chunk
NCHUNK = NTOK // CN
KT = D // 128           # k-tiles over model dim
QC = 512                # phase-2 query chunk
SCALE = 1.0 / math.sqrt(HD)

_CACHE = {}


def _build_nc():
    import concourse.tile as tile
    import concourse.mybir as mybir
    from concourse import bacc
    from contextlib import ExitStack

    f32 = mybir.dt.float32
    f32r = mybir.dt.float32r
    bf16 = mybir.dt.bfloat16
    MULT = mybir.AluOpType.mult
    ADD = mybir.AluOpType.add
    EXP = mybir.ActivationFunctionType.Exp
    IDENT = mybir.ActivationFunctionType.Identity

    nc = bacc.Bacc("TRN2", target_bir_lowering=False, debug=False,
                   num_devices=NCORES)

    xT = nc.dram_tensor("xT", [D, NTOK], bf16, kind="ExternalInput")
    wq = nc.dram_tensor("wq", [D, CW], bf16, kind="ExternalInput")
    wk = nc.dram_tensor("wk", [D, CW], bf16, kind="ExternalInput")
    wv = nc.dram_tensor("wv", [D, CW], bf16, kind="ExternalInput")
    wo = nc.dram_tensor("wo", [CW, D], bf16, kind="ExternalInput")
    qb = nc.dram_tensor("qb", [128, HLOC], f32, kind="ExternalInput")
    kb = nc.dram_tensor("kb", [128, HLOC], f32, kind="ExternalInput")
    vb = nc.dram_tensor("vb", [128, CW], f32, kind="ExternalInput")
    cos = nc.dram_tensor("cos", [128, S], f32, kind="ExternalInput")
    sin = nc.dram_tensor("sin", [128, S], f32, kind="ExternalInput")
    swapm = nc.dram_tensor("swapm", [128, 128], bf16, kind="ExternalInput")
    triu = nc.dram_tensor("triu", [128, 128], f32, kind="ExternalInput")

    out_p = nc.dram_tensor("out_p", [NTOK, D], bf16, kind="ExternalOutput")
    k_out = nc.dram_tensor("k_out", [CW, NTOK], f32, kind="ExternalOutput")
    v_out = nc.dram_tensor("v_out", [NTOK, CW], f32, kind="ExternalOutput")

    def r(ap):
        return ap.bitcast(f32r)

    with tile.TileContext(nc) as tc, ExitStack() as ctx:
        res = ctx.enter_context(tc.tile_pool(name="res", bufs=1))
        qres = [res.tile([128, NTOK], bf16, tag=f"qres{h}", name=f"qres{h}") for h in range(HLOC)]
        kres = [res.tile([128, NTOK], bf16, tag=f"kres{h}", name=f"kres{h}") for h in range(HLOC)]
        vres = [res.tile([128, NTOK], bf16, tag=f"vres{h}", name=f"vres{h}") for h in range(HLOC)]
        cxres = [res.tile([128, NTOK], bf16, tag=f"cxres{h}", name=f"cxres{h}") for h in range(HLOC)]

        cpool = ctx.enter_context(tc.tile_pool(name="consts", bufs=1))
        swapm_sb = cpool.tile([128, 128], bf16, tag="swapm")
        triu_sb = cpool.tile([128, 128], f32, tag="triu")
        qb_sb = cpool.tile([128, HLOC], f32, tag="qb")
        kb_sb = cpool.tile([128, HLOC], f32, tag="kb")
        vb_sb = cpool.tile([128, CW], f32, tag="vb")
        cos_sb = cpool.tile([128, S], f32, tag="cos")
        sin_sb = cpool.tile([128, S], f32, tag="sin")
        ones_mat = cpool.tile([128, 128], bf16, tag="onesm")
        for t, src in ((swapm_sb, swapm), (triu_sb, triu), (qb_sb, qb),
                       (kb_sb, kb), (vb_sb, vb), (cos_sb, cos), (sin_sb, sin)):
            nc.sync.dma_start(t[:], src[:, :])
        nc.vector.memset(ones_mat[:], 1.0)

        # ---------------- phase 1: projections + RoPE ----------------
        with tc.tile_pool(name="wts", bufs=1) as wpool, \
             tc.tile_pool(name="xin", bufs=2) as xpool, \
             tc.tile_pool(name="stg", bufs=2) as spool, \
             tc.tile_pool(name="p1ps", bufs=2, space="PSUM") as p1ps, \
             tc.tile_pool(name="swps", bufs=2, space="PSUM") as swps:
            wq_sb = wpool.tile([128, KT * CW], bf16, tag="wq")
            wk_sb = wpool.tile([128, KT * CW], bf16, tag="wk")
            wv_sb = wpool.tile([128, KT * CW], bf16, tag="wv")
            for t, src in ((wq_sb, wq), (wk_sb, wk), (wv_sb, wv)):
                nc.sync.dma_start(
                    t[:].rearrange("p (kt m) -> p kt m", m=CW),
                    src[:, :].rearrange("(kt p) m -> p kt m", p=128))

            for c in range(NCHUNK):
                t0 = c * CN
                pos = t0 % S
                xt = xpool.tile([128, KT * CN], bf16, tag="xt")
                nc.sync.dma_start(
                    xt[:].rearrange("p (kt t) -> p kt t", t=CN),
                    xT[:, t0:t0 + CN].rearrange("(kt p) t -> p kt t", p=128))
                x3 = xt[:].rearrange("p (kt t) -> p kt t", t=CN)

                for h in range(HLOC):
                    for which, w_sb, b_sb in (("q", wq_sb, qb_sb),
                                              ("k", wk_sb, kb_sb)):
                        p_ps = p1ps.tile([128, CN], f32, tag="proj")
                        for kt in range(KT):
                            nc.tensor.matmul(
                                p_ps[:],
                                w_sb[:, kt * CW + h * HD:kt * CW + (h + 1) * HD],
                                x3[:, kt, :],
                                start=(kt == 0), stop=(kt == KT - 1))
                        p_sb = spool.tile([128, CN], bf16, tag="psb")
                        nc.scalar.activation(p_sb[:], p_ps[:], IDENT,
                                             bias=b_sb[:, h:h + 1])
                        sw_ps = swps.tile([128, CN], f32, tag="swap")
                        nc.tensor.matmul(sw_ps[:], swapm_sb[:], p_sb[:],
                                         start=True, stop=True)
                        t1 = spool.tile([128, CN], f32, tag="t1")
                        nc.vector.tensor_tensor(t1[:], p_sb[:],
                                                cos_sb[:, pos:pos + CN], MULT)
                        t2 = spool.tile([128, CN], f32, tag="t2")
                        nc.vector.tensor_tensor(t2[:], sw_ps[:],
                                                sin_sb[:, pos:pos + CN], MULT)
                        if which == "q":
                            nc.vector.tensor_tensor(qres[h][:, t0:t0 + CN],
                                                    t1[:], t2[:], ADD)
                        else:
                            krot = spool.tile([128, CN], f32, tag="krot")
                            nc.vector.tensor_tensor(krot[:], t1[:], t2[:], ADD)
                            nc.sync.dma_start(
                                k_out[h * HD:(h + 1) * HD, t0:t0 + CN], krot[:])
                            nc.scalar.copy(kres[h][:, t0:t0 + CN], krot[:])

                for ts in range(CN // 128):
                    v_ps = p1ps.tile([128, CW], f32, tag="vproj")
                    for kt in range(KT):
                        nc.tensor.matmul(
                            v_ps[:],
                            x3[:, kt, ts * 128:(ts + 1) * 128],
                            wv_sb[:, kt * CW:(kt + 1) * CW],
                            start=(kt == 0), stop=(kt == KT - 1))
                    v_sb = spool.tile([128, CW], f32, tag="vsb")
                    nc.vector.tensor_tensor(v_sb[:], v_ps[:], vb_sb[:], ADD)
                    nc.sync.dma_start(
                        v_out[t0 + ts * 128:t0 + (ts + 1) * 128, :], v_sb[:])
                    bt = c * (CN // 128) + ts
                    for h in range(HLOC):
                        nc.scalar.copy(vres[h][:, bt * 128:(bt + 1) * 128],
                                       v_sb[:, h * HD:(h + 1) * HD])

        # ------------ phase 2: causal attention + output projection ------------
        with tc.tile_pool(name="wo", bufs=1) as wop, \
             tc.tile_pool(name="osb", bufs=4) as osb, \
             tc.tile_pool(name="pt", bufs=6) as ptpool, \
             tc.tile_pool(name="p2sb", bufs=3) as p2sb, \
             tc.tile_pool(name="stps", bufs=3, space="PSUM") as stps, \
             tc.tile_pool(name="ctxps", bufs=2, space="PSUM") as ctxps, \
             tc.tile_pool(name="rsps", bufs=1, space="PSUM") as rsps, \
             tc.tile_pool(name="ops", bufs=2, space="PSUM") as ops:
            wo_sb = [wop.tile([128, D], bf16, tag=f"wo{h}", name=f"wo{h}")
                     for h in range(HLOC)]
            for h in range(HLOC):
                nc.sync.dma_start(wo_sb[h][:], wo[h * HD:(h + 1) * HD, :])
            for b in range(B):
                for qc in range(S // QC):
                    for h in range(HLOC):
                        g0 = b * S + qc * QC
                        nkt = (qc + 1) * (QC // 128)
                        ctx_ps = ctxps.tile([128, QC], f32, tag="ctx")
                        rs_ps = rsps.tile([128, QC], f32, tag="rs")
                        for kt in range(nkt):
                            gk = b * S + kt * 128
                            # causal: this k-tile only contributes to q >= q0v
                            j0 = max(kt - qc * (QC // 128), 0)
                            q0v = j0 * 128
                            st_ps = stps.tile([128, QC], f32, tag="st")
                            nc.tensor.matmul(st_ps[:, q0v:QC],
                                             kres[h][:, gk:gk + 128],
                                             qres[h][:, g0 + q0v:g0 + QC],
                                             start=True, stop=True)
                            pt = ptpool.tile([128, QC], bf16, tag="pt")
                            if kt - qc * (QC // 128) >= 0:
                                nc.vector.tensor_tensor(
                                    st_ps[:, q0v:q0v + 128],
                                    st_ps[:, q0v:q0v + 128],
                                    triu_sb[:], ADD)
                            nc.scalar.activation(pt[:, q0v:QC],
                                                 st_ps[:, q0v:QC],
                                                 EXP, scale=SCALE)
                            nc.tensor.matmul(rs_ps[:, q0v:QC], ones_mat[:],
                                             pt[:, q0v:QC],
                                             start=(kt == 0),
                                             stop=(kt == nkt - 1))
                            bt = b * (S // 128) + kt
                            nc.tensor.matmul(ctx_ps[:, q0v:QC],
                                             vres[h][:, bt * 128:(bt + 1) * 128],
                                             pt[:, q0v:QC], start=(kt == 0),
                                             stop=(kt == nkt - 1))
                        rec_sb = p2sb.tile([128, QC], f32, tag="rec")
                        nc.vector.reciprocal_approx_fast(rec_sb[:], rs_ps[:])
                        nc.vector.tensor_tensor(cxres[h][:, g0:g0 + QC],
                                                ctx_ps[:], rec_sb[:], MULT)
                    # output projection for this (b, qc) token range
                    for i in range(QC // 128):
                        tt = (b * S + qc * QC) // 128 + i
                        o_sb = osb.tile([128, D], bf16, tag="osb")
                        for ncn in range(D // 512):
                            o_ps = ops.tile([128, 512], f32, tag="o")
                            for h in range(HLOC):
                                nc.tensor.matmul(
                                    o_ps[:],
                                    cxres[h][:, tt * 128:(tt + 1) * 128],
                                    wo_sb[h][:, ncn * 512:(ncn + 1) * 512],
                                    start=(h == 0), stop=(h == HLOC - 1))
                            if (tt + ncn) % 2 == 0:
                                nc.scalar.copy(o_sb[:, ncn * 512:(ncn + 1) * 512],
                                               o_ps[:])
                            else:
                                nc.vector.tensor_copy(
                                    o_sb[:, ncn * 512:(ncn + 1) * 512], o_ps[:])
                        nc.sync.dma_start(out_p[tt * 128:(tt + 1) * 128, :],
                                          o_sb[:])

    nc.compile()
    return nc


def get_nc():
    if "nc" not in _CACHE:
        _CACHE["nc"] = _build_nc()
    return _CACHE["nc"]


def prep_in_maps(inputs):
    x = np.asarray(inputs["x"], dtype=np.float32)
    fc = np.asarray(inputs["freqs_cos"], dtype=np.float32)
    fs = np.asarray(inputs["freqs_sin"], dtype=np.float32)
    wq_w = np.asarray(inputs["wq_w"], dtype=np.float32)
    wk_w = np.asarray(inputs["wk_w"], dtype=np.float32)
    wv_w = np.asarray(inputs["wv_w"], dtype=np.float32)
    wo_w = np.asarray(inputs["wo_w"], dtype=np.float32)
    wq_b = np.asarray(inputs["wq_b"], dtype=np.float32)
    wk_b = np.asarray(inputs["wk_b"], dtype=np.float32)
    wv_b = np.asarray(inputs["wv_b"], dtype=np.float32)

    import ml_dtypes
    bf = ml_dtypes.bfloat16
    xT = np.ascontiguousarray(x.reshape(NTOK, D).T).astype(bf)

    cosE = np.empty((128, S), np.float32)
    cosE[0::2] = fc.T
    cosE[1::2] = fc.T
    sinE = np.empty((128, S), np.float32)
    sinE[0::2] = -fs.T
    sinE[1::2] = fs.T

    swapm = np.zeros((128, 128), np.float32)
    swapm[np.arange(128) ^ 1, np.arange(128)] = 1.0

    ar = np.arange(128)
    triu = np.where(ar[:, None] > ar[None, :], np.float32(-1e30),
                    np.float32(0.0)).astype(np.float32)

    in_maps = []
    for c in range(NCORES):
        cs = slice(c * CW, (c + 1) * CW)
        in_maps.append({
            "xT": xT,
            "wq": np.ascontiguousarray(wq_w[cs, :].T).astype(bf),
            "wk": np.ascontiguousarray(wk_w[cs, :].T).astype(bf),
            "wv": np.ascontiguousarray(wv_w[cs, :].T).astype(bf),
            "wo": np.ascontiguousarray(wo_w[:, cs].T).astype(bf),
            "qb": np.ascontiguousarray(wq_b[cs].reshape(HLOC, 128).T),
            "kb": np.ascontiguousarray(wk_b[cs].reshape(HLOC, 128).T),
            "vb": np.broadcast_to(wv_b[cs], (128, CW)).copy(),
            "cos": cosE,
            "sin": sinE,
            "swapm": swapm.astype(bf),
            "triu": triu,
        })
    return in_maps


def postprocess(results, wo_b):
    wo_b = np.asarray(wo_b, dtype=np.float32)
    out = results[0]["out_p"].astype(np.float32)
    for c in range(1, NCORES):
        out += results[c]["out_p"].astype(np.float32)
    out += wo_b
    out = out.reshape(B, S, D)

    k_full = np.empty((B, H, S, HD), np.float32)
    v_full = np.empty((B, H, S, HD), np.float32)
    for c in range(NCORES):
        kc = results[c]["k_out"].reshape(HLOC, HD, B, S).transpose(2, 0, 3, 1)
        k_full[:, c * HLOC:(c + 1) * HLOC] = kc
        vc = results[c]["v_out"].reshape(B, S, HLOC, HD).transpose(0, 2, 1, 3)
        v_full[:, c * HLOC:(c + 1) * HLOC] = vc
    return out, k_full, v_full


def run(inputs, trace=False):
    from concourse.bass_utils import run_bass_kernel_spmd
    nc = get_nc()
    in_maps = prep_in_maps(inputs)
    res = run_bass_kernel_spmd(nc, in_maps, core_ids=list(range(NCORES)),
                               trace=trace)
    out, k_full, v_full = postprocess(res.results, inputs["wo_b"])
    return (out, k_full, v_full), res


def kernel(**inputs):
    (out, k_full, v_full), _ = run(inputs, trace=False)
    return out, k_full, v_full


# revision 11
# speedup vs baseline: 1.0224x; 1.0224x over previous
"""Multi-head causal attention (B=2, S=2048, D=2048, H=16) on 8 TRN2 NeuronCores.

Sharding: tensor-parallel over heads — each core computes 2 heads end-to-end
(QKV projection columns, RoPE, causal attention, wo projection rows). The wo
partial outputs are summed on the host (row-parallel matmul unshard).

Device layouts (per core):
  xT   [2048, 4096]  x transposed: model-dim on partitions, tokens free
  Q/K  computed transposed [d, tok] so attention needs no activations transpose:
       S^T[k,q] = (K^T slice).T-matmul(Q^T slice); softmax runs on ACT with the
       row sum realized as a ones-vector matmul on the PE; P^T feeds P@V
       directly, producing ctx^T[d, tok] which is exactly the lhsT the wo
       projection needs.
  RoPE pair-swap (interleaved rotary) is a 128x128 permutation matmul on PE.
"""

import math
import sys

sys.path.insert(0, "/opt/trn_rl_repo")

import numpy as np

B = 2
S = 2048
D = 2048
H = 16
HD = 128
NCORES = 8
HLOC = H // NCORES      # heads per core
CW = HLOC * HD          # per-core projection width (256)
NTOK = B * S            # 4096
CN = 512                # phase-1 token chunk
NCHUNK = NTOK // CN
KT = D // 128           # k-tiles over model dim
QC = 512                # phase-2 query chunk
SCALE = 1.0 / math.sqrt(HD)

_CACHE = {}


def _build_nc():
    import concourse.tile as tile
    import concourse.mybir as mybir
    from concourse import bacc
    from contextlib import ExitStack

    f32 = mybir.dt.float32
    f32r = mybir.dt.float32r
    bf16 = mybir.dt.bfloat16
    MULT = mybir.AluOpType.mult
    ADD = mybir.AluOpType.add
    EXP = mybir.ActivationFunctionType.Exp
    IDENT = mybir.ActivationFunctionType.Identity

    nc = bacc.Bacc("TRN2", target_bir_lowering=False, debug=False,
                   num_devices=NCORES)

    xT = nc.dram_tensor("xT", [D, NTOK], bf16, kind="ExternalInput")
    wq = nc.dram_tensor("wq", [D, CW], bf16, kind="ExternalInput")
    wk = nc.dram_tensor("wk", [D, CW], bf16, kind="ExternalInput")
    wv = nc.dram_tensor("wv", [D, CW], bf16, kind="ExternalInput")
    wo = nc.dram_tensor("wo", [CW, D], bf16, kind="ExternalInput")
    qb = nc.dram_tensor("qb", [128, HLOC], f32, kind="ExternalInput")
    kb = nc.dram_tensor("kb", [128, HLOC], f32, kind="ExternalInput")
    vb = nc.dram_tensor("vb", [128, CW], f32, kind="ExternalInput")
    cos = nc.dram_tensor("cos", [128, S], f32, kind="ExternalInput")
    sin = nc.dram_tensor("sin", [128, S], f32, kind="ExternalInput")
    swapm = nc.dram_tensor("swapm", [128, 128], bf16, kind="ExternalInput")
    triu = nc.dram_tensor("triu", [128, 128], f32, kind="ExternalInput")

    out_p = nc.dram_tensor("out_p", [NTOK, D], bf16, kind="ExternalOutput")
    k_out = nc.dram_tensor("k_out", [CW, NTOK], f32, kind="ExternalOutput")
    v_out = nc.dram_tensor("v_out", [NTOK, CW], f32, kind="ExternalOutput")

    def r(ap):
        return ap.bitcast(f32r)

    with tile.TileContext(nc) as tc, ExitStack() as ctx:
        res = ctx.enter_context(tc.tile_pool(name="res", bufs=1))
        qres = [res.tile([128, NTOK], bf16, tag=f"qres{h}", name=f"qres{h}") for h in range(HLOC)]
        kres = [res.tile([128, NTOK], bf16, tag=f"kres{h}", name=f"kres{h}") for h in range(HLOC)]
        vres = [res.tile([128, NTOK], bf16, tag=f"vres{h}", name=f"vres{h}") for h in range(HLOC)]
        cxres = [res.tile([128, NTOK], bf16, tag=f"cxres{h}", name=f"cxres{h}") for h in range(HLOC)]

        cpool = ctx.enter_context(tc.tile_pool(name="consts", bufs=1))
        swapm_sb = cpool.tile([128, 128], bf16, tag="swapm")
        triu_sb = cpool.tile([128, 128], f32, tag="triu")
        qb_sb = cpool.tile([128, HLOC], f32, tag="qb")
        kb_sb = cpool.tile([128, HLOC], f32, tag="kb")
        vb_sb = cpool.tile([128, CW], f32, tag="vb")
        cos_sb = cpool.tile([128, S], f32, tag="cos")
        sin_sb = cpool.tile([128, S], f32, tag="sin")
        ones_mat = cpool.tile([128, 128], bf16, tag="onesm")
        for t, src in ((swapm_sb, swapm), (triu_sb, triu), (qb_sb, qb),
                       (kb_sb, kb), (vb_sb, vb), (cos_sb, cos), (sin_sb, sin)):
            nc.sync.dma_start(t[:], src[:, :])
        nc.vector.memset(ones_mat[:], 1.0)

        # ---------------- phase 1: projections + RoPE ----------------
        with tc.tile_pool(name="wts", bufs=1) as wpool, \
             tc.tile_pool(name="xin", bufs=2) as xpool, \
             tc.tile_pool(name="stg", bufs=2) as spool, \
             tc.tile_pool(name="p1ps", bufs=2, space="PSUM") as p1ps, \
             tc.tile_pool(name="swps", bufs=2, space="PSUM") as swps:
            wq_sb = wpool.tile([128, KT * CW], bf16, tag="wq")
            wk_sb = wpool.tile([128, KT * CW], bf16, tag="wk")
            wv_sb = wpool.tile([128, KT * CW], bf16, tag="wv")
            for t, src in ((wq_sb, wq), (wk_sb, wk), (wv_sb, wv)):
                nc.sync.dma_start(
                    t[:].rearrange("p (kt m) -> p kt m", m=CW),
                    src[:, :].rearrange("(kt p) m -> p kt m", p=128))

            for c in range(NCHUNK):
                t0 = c * CN
                pos = t0 % S
                xt = xpool.tile([128, KT * CN], bf16, tag="xt")
                for kq in range(4):
                    ks, ke = kq * (KT // 4), (kq + 1) * (KT // 4)
                    nc.sync.dma_start(
                        xt[:, ks * CN:ke * CN].rearrange(
                            "p (kt t) -> p kt t", t=CN),
                        xT[ks * 128:ke * 128, t0:t0 + CN].rearrange(
                            "(kt p) t -> p kt t", p=128))
                x3 = xt[:].rearrange("p (kt t) -> p kt t", t=CN)

                for h in range(HLOC):
                    for which, w_sb, b_sb in (("q", wq_sb, qb_sb),
                                              ("k", wk_sb, kb_sb)):
                        p_ps = p1ps.tile([128, CN], f32, tag="proj")
                        for kt in range(KT):
                            nc.tensor.matmul(
                                p_ps[:],
                                w_sb[:, kt * CW + h * HD:kt * CW + (h + 1) * HD],
                                x3[:, kt, :],
                                start=(kt == 0), stop=(kt == KT - 1))
                        p_sb = spool.tile([128, CN], bf16, tag="psb")
                        nc.scalar.activation(p_sb[:], p_ps[:], IDENT,
                                             bias=b_sb[:, h:h + 1])
                        sw_ps = swps.tile([128, CN], f32, tag="swap")
                        nc.tensor.matmul(sw_ps[:], swapm_sb[:], p_sb[:],
                                         start=True, stop=True)
                        t1 = spool.tile([128, CN], f32, tag="t1")
                        nc.vector.tensor_tensor(t1[:], p_sb[:],
                                                cos_sb[:, pos:pos + CN], MULT)
                        t2 = spool.tile([128, CN], f32, tag="t2")
                        nc.vector.tensor_tensor(t2[:], sw_ps[:],
                                                sin_sb[:, pos:pos + CN], MULT)
                        if which == "q":
                            nc.vector.tensor_tensor(qres[h][:, t0:t0 + CN],
                                                    t1[:], t2[:], ADD)
                        else:
                            krot = spool.tile([128, CN], f32, tag="krot")
                            nc.vector.tensor_tensor(krot[:], t1[:], t2[:], ADD)
                            nc.sync.dma_start(
                                k_out[h * HD:(h + 1) * HD, t0:t0 + CN], krot[:])
                            nc.scalar.copy(kres[h][:, t0:t0 + CN], krot[:])

                for ts in range(CN // 128):
                    v_ps = p1ps.tile([128, CW], f32, tag="vproj")
                    for kt in range(KT):
                        nc.tensor.matmul(
                            v_ps[:],
                            x3[:, kt, ts * 128:(ts + 1) * 128],
                            wv_sb[:, kt * CW:(kt + 1) * CW],
                            start=(kt == 0), stop=(kt == KT - 1))
                    v_sb = spool.tile([128, CW], f32, tag="vsb")
                    nc.vector.tensor_tensor(v_sb[:], v_ps[:], vb_sb[:], ADD)
                    nc.sync.dma_start(
                        v_out[t0 + ts * 128:t0 + (ts + 1) * 128, :], v_sb[:])
                    bt = c * (CN // 128) + ts
                    for h in range(HLOC):
                        nc.scalar.copy(vres[h][:, bt * 128:(bt + 1) * 128],
                                       v_sb[:, h * HD:(h + 1) * HD])

        # ------------ phase 2: causal attention + output projection ------------
        with tc.tile_pool(name="wo", bufs=1) as wop, \
             tc.tile_pool(name="osb", bufs=4) as osb, \
             tc.tile_pool(name="pt", bufs=8) as ptpool, \
             tc.tile_pool(name="p2sb", bufs=3) as p2sb, \
             tc.tile_pool(name="stps", bufs=3, space="PSUM") as stps, \
             tc.tile_pool(name="ctxps", bufs=2, space="PSUM") as ctxps, \
             tc.tile_pool(name="rsps", bufs=1, space="PSUM") as rsps, \
             tc.tile_pool(name="ops", bufs=2, space="PSUM") as ops:
            wo_sb = [wop.tile([128, D], bf16, tag=f"wo{h}", name=f"wo{h}")
                     for h in range(HLOC)]
            for h in range(HLOC):
                nc.sync.dma_start(wo_sb[h][:], wo[h * HD:(h + 1) * HD, :])
            for b in range(B):
                for qc in range(S // QC):
                    for h in range(HLOC):
                        g0 = b * S + qc * QC
                        nkt = (qc + 1) * (QC // 128)
                        ctx_ps = ctxps.tile([128, QC], f32, tag="ctx")
                        rs_ps = rsps.tile([128, QC], f32, tag="rs")
                        for kt in range(nkt):
                            gk = b * S + kt * 128
                            # causal: this k-tile only contributes to q >= q0v
                            j0 = max(kt - qc * (QC // 128), 0)
                            q0v = j0 * 128
                            st_ps = stps.tile([128, QC], f32, tag="st")
                            nc.tensor.matmul(st_ps[:, q0v:QC],
                                             kres[h][:, gk:gk + 128],
                                             qres[h][:, g0 + q0v:g0 + QC],
                                             start=True, stop=True)
                            pt = ptpool.tile([128, QC], bf16, tag="pt")
                            if kt - qc * (QC // 128) >= 0:
                                nc.vector.tensor_tensor(
                                    st_ps[:, q0v:q0v + 128],
                                    st_ps[:, q0v:q0v + 128],
                                    triu_sb[:], ADD)
                            nc.scalar.activation(pt[:, q0v:QC],
                                                 st_ps[:, q0v:QC],
                                                 EXP, scale=SCALE)
                            nc.tensor.matmul(rs_ps[:, q0v:QC], ones_mat[:],
                                             pt[:, q0v:QC],
                                             start=(kt == 0),
                                             stop=(kt == nkt - 1))
                            bt = b * (S // 128) + kt
                            nc.tensor.matmul(ctx_ps[:, q0v:QC],
                                             vres[h][:, bt * 128:(bt + 1) * 128],
                                             pt[:, q0v:QC], start=(kt == 0),
                                             stop=(kt == nkt - 1))
                        rec_sb = p2sb.tile([128, QC], f32, tag="rec")
                        nc.vector.reciprocal_approx_fast(rec_sb[:], rs_ps[:])
                        nc.vector.tensor_tensor(cxres[h][:, g0:g0 + QC],
                                                ctx_ps[:], rec_sb[:], MULT)
                    # output projection for this (b, qc) token range
                    for i in range(QC // 128):
                        tt = (b * S + qc * QC) // 128 + i
                        o_sb = osb.tile([128, D], bf16, tag="osb")
                        for ncn in range(D // 512):
                            o_ps = ops.tile([128, 512], f32, tag="o")
                            for h in range(HLOC):
                                nc.tensor.matmul(
                                    o_ps[:],
                                    cxres[h][:, tt * 128:(tt + 1) * 128],
                                    wo_sb[h][:, ncn * 512:(ncn + 1) * 512],
                                    start=(h == 0), stop=(h == HLOC - 1))
                            if (tt + ncn) % 2 == 0:
                                nc.scalar.copy(o_sb[:, ncn * 512:(ncn + 1) * 512],
                                               o_ps[:])
                            else:
                                nc.vector.tensor_copy(
                                    o_sb[:, ncn * 512:(ncn + 1) * 512], o_ps[:])
                        nc.sync.dma_start(out_p[tt * 128:(tt + 1) * 128, :],
                                          o_sb[:])

    nc.compile()
    return nc


def get_nc():
    if "nc" not in _CACHE:
        _CACHE["nc"] = _build_nc()
    return _CACHE["nc"]


def prep_in_maps(inputs):
    x = np.asarray(inputs["x"], dtype=np.float32)
    fc = np.asarray(inputs["freqs_cos"], dtype=np.float32)
    fs = np.asarray(inputs["freqs_sin"], dtype=np.float32)
    wq_w = np.asarray(inputs["wq_w"], dtype=np.float32)
    wk_w = np.asarray(inputs["wk_w"], dtype=np.float32)
    wv_w = np.asarray(inputs["wv_w"], dtype=np.float32)
    wo_w = np.asarray(inputs["wo_w"], dtype=np.float32)
    wq_b = np.asarray(inputs["wq_b"], dtype=np.float32)
    wk_b = np.asarray(inputs["wk_b"], dtype=np.float32)
    wv_b = np.asarray(inputs["wv_b"], dtype=np.float32)

    import ml_dtypes
    bf = ml_dtypes.bfloat16
    xT = np.ascontiguousarray(x.reshape(NTOK, D).T).astype(bf)

    cosE = np.empty((128, S), np.float32)
    cosE[0::2] = fc.T
    cosE[1::2] = fc.T
    sinE = np.empty((128, S), np.float32)
    sinE[0::2] = -fs.T
    sinE[1::2] = fs.T

    swapm = np.zeros((128, 128), np.float32)
    swapm[np.arange(128) ^ 1, np.arange(128)] = 1.0

    ar = np.arange(128)
    triu = np.where(ar[:, None] > ar[None, :], np.float32(-1e30),
                    np.float32(0.0)).astype(np.float32)

    in_maps = []
    for c in range(NCORES):
        cs = slice(c * CW, (c + 1) * CW)
        in_maps.append({
            "xT": xT,
            "wq": np.ascontiguousarray(wq_w[cs, :].T).astype(bf),
            "wk": np.ascontiguousarray(wk_w[cs, :].T).astype(bf),
            "wv": np.ascontiguousarray(wv_w[cs, :].T).astype(bf),
            "wo": np.ascontiguousarray(wo_w[:, cs].T).astype(bf),
            "qb": np.ascontiguousarray(wq_b[cs].reshape(HLOC, 128).T),
            "kb": np.ascontiguousarray(wk_b[cs].reshape(HLOC, 128).T),
            "vb": np.broadcast_to(wv_b[cs], (128, CW)).copy(),
            "cos": cosE,
            "sin": sinE,
            "swapm": swapm.astype(bf),
            "triu": triu,
        })
    return in_maps


def postprocess(results, wo_b):
    wo_b = np.asarray(wo_b, dtype=np.float32)
    out = results[0]["out_p"].astype(np.float32)
    for c in range(1, NCORES):
        out += results[c]["out_p"].astype(np.float32)
    out += wo_b
    out = out.reshape(B, S, D)

    k_full = np.empty((B, H, S, HD), np.float32)
    v_full = np.empty((B, H, S, HD), np.float32)
    for c in range(NCORES):
        kc = results[c]["k_out"].reshape(HLOC, HD, B, S).transpose(2, 0, 3, 1)
        k_full[:, c * HLOC:(c + 1) * HLOC] = kc
        vc = results[c]["v_out"].reshape(B, S, HLOC, HD).transpose(0, 2, 1, 3)
        v_full[:, c * HLOC:(c + 1) * HLOC] = vc
    return out, k_full, v_full


def run(inputs, trace=False):
    from concourse.bass_utils import run_bass_kernel_spmd
    nc = get_nc()
    in_maps = prep_in_maps(inputs)
    res = run_bass_kernel_spmd(nc, in_maps, core_ids=list(range(NCORES)),
                               trace=trace)
    out, k_full, v_full = postprocess(res.results, inputs["wo_b"])
    return (out, k_full, v_full), res


def kernel(**inputs):
    (out, k_full, v_full), _ = run(inputs, trace=False)
    return out, k_full, v_full
